# revision 11
# baseline (speedup 1.0000x reference)
"""Trainium2 Bass kernel for GRU decoder (nn_RNNDecoder).

B=32, S=128, H=512, V=32000. Sharding: data-parallel recurrence (4
batches/core) + vocab-parallel output GEMM (4000 rows/core) with an
AllGather of the hidden states in between. All GEMMs run in fp16 with
fp32 psum accumulation and an fp32 master copy of h.
"""

import sys

sys.path.insert(0, "/opt/trn_rl_repo")

import json as _json
from contextlib import ExitStack
from itertools import count as _count

import numpy as np

import concourse.bass as bass
import concourse.tile as tile
from concourse import mybir
from concourse.masks import make_identity

FP32 = mybir.dt.float32
FP16 = mybir.dt.float16
I32 = mybir.dt.int32

B, S, H, V = 32, 128, 512, 32000
NCORES = 8
BPC = B // NCORES  # batches per core
VPC = V // NCORES  # vocab rows per core
TOK = BPC * S  # tokens per core
NK = H // 128  # 4 hidden chunks
NM = 3 * H // 128  # 12 gate chunks
VHALF = 2000  # vocab per psum group
NVN = 4  # psum tiles per group ([128,500] each)

# ---------------------------------------------------------------------------
# Workaround for this container's walrus codegen: instructions whose ISA
# struct has fewer sync-wait slots than the tile framework emits fail with
# "Too many sync wait commands". Split excess waits onto NoOp carriers on
# the same engine (in-order execution preserves semantics).
_uid = _count()


def _fix_bir_sync(bir_json, limit=1):
    m = _json.loads(bir_json)
    for fn in m["functions"]:
        for blk in fn["blocks"]:
            out = []
            for inst in blk["instructions"]:
                si = inst.get("sync_info") or {}
                waits = si.get("on_wait") or []
                if len(waits) > limit:
                    keep = waits[len(waits) - limit :]
                    excess = waits[: len(waits) - limit]
                    for w in excess:
                        out.append(
                            {
                                "engine": inst["engine"],
                                "ins": [],
                                "outs": [],
                                "name": f"syncfix-{next(_uid)}",
                                "opcode": "NoOp",
                                "sync_info": {"on_update": [], "on_wait": [w]},
                            }
                        )
                    si = dict(si)
                    si["on_wait"] = keep
                    inst["sync_info"] = si
                out.append(inst)
            blk["instructions"] = out
    return _json.dumps(m).encode()


_installed = False


def _install_syncfix():
    global _installed
    if _installed:
        return
    from concourse import bass_utils, bass2jax

    orig = bass_utils.compile_bir_kernel

    def patched(bir_json, tmpdir, neff_name="file.neff"):
        return orig(_fix_bir_sync(bir_json), tmpdir, neff_name)

    bass_utils.compile_bir_kernel = patched
    bass2jax.compile_bir_kernel = patched
    _installed = True


# ---------------------------------------------------------------------------


def build_nc(nsteps=S, reps=1, timing=False):
    # timing=True shrinks the host-transfer surface (tiny emb upload, tiny
    # output download) without changing device-side work in the timed loop.
    nc = bass.Bass()
    idx_t = nc.declare_dram_parameter("idx_t", [S, BPC], I32, isOutput=False)
    emb = nc.declare_dram_parameter("emb", [128 if timing else V, H], FP16, isOutput=False)
    h0_t = nc.declare_dram_parameter("h0_t", [128, NK * BPC], FP32, isOutput=False)
    w_ih = nc.declare_dram_parameter("w_ih", [128, NK * NM * 128], FP16, isOutput=False)
    w_hh = nc.declare_dram_parameter("w_hh", [128, NK * NM * 128], FP16, isOutput=False)
    b_comb = nc.declare_dram_parameter("b_comb", [128, NM], FP32, isOutput=False)
    b_hn = nc.declare_dram_parameter("b_hn", [128, NK * BPC], FP32, isOutput=False)
    w_out = nc.declare_dram_parameter("w_out", [128, NK * VPC], FP16, isOutput=False)
    b_rep = nc.declare_dram_parameter("b_rep", [128, VPC], FP32, isOutput=False)
    out = nc.declare_dram_parameter(
        "out", [256 if timing else B * S, VPC], FP32, isOutput=True
    )

    with tile.TileContext(nc) as tc, ExitStack() as ctx:
        sb = ctx.enter_context(tc.tile_pool(name="sb", bufs=1))
        dr = ctx.enter_context(tc.tile_pool(name="dr", bufs=1, space="DRAM"))

        # persistent SBUF state
        idx_sb = sb.tile([S, BPC], I32)
        h0_sb = sb.tile([128, NK * BPC], FP32)
        wih_sb = sb.tile([128, NK, NM, 128], FP16)
        whh_sb = sb.tile([128, NK, NM, 128], FP16)
        bcomb_sb = sb.tile([128, NM], FP32)
        bhn_sb = sb.tile([128, NK * BPC], FP32)
        wout_sb = sb.tile([128, NK, VPC], FP16)
        brep_sb = sb.tile([128, VPC], FP32)
        nc.sync.dma_start(out=idx_sb[:], in_=idx_t[:])
        nc.sync.dma_start(out=h0_sb[:], in_=h0_t[:])
        nc.sync.dma_start(out=wih_sb[:], in_=w_ih[:])
        nc.sync.dma_start(out=whh_sb[:], in_=w_hh[:])
        nc.sync.dma_start(out=bcomb_sb[:], in_=b_comb[:])
        nc.sync.dma_start(out=bhn_sb[:], in_=b_hn[:])
        nc.sync.dma_start(out=wout_sb[:], in_=w_out[:])
        nc.sync.dma_start(out=brep_sb[:], in_=b_rep[:])

        ident = sb.tile([128, 128], FP16)
        make_identity(nc, ident[:])

        xgT = sb.tile([128, NM, BPC, S], FP32)
        hid32 = sb.tile([128, NK, BPC, S + 1], FP32)
        hid16 = sb.tile([128, NK, BPC, S + 1], FP16)

        # gate-math temporaries (reused every step)
        d1 = sb.tile([128, 2 * NK * BPC], FP32)
        tau = sb.tile([128, 2 * NK * BPC], FP32)
        hnb = sb.tile([128, NK * BPC], FP32)
        u_t = sb.tile([128, NK * BPC], FP32)
        v_t = sb.tile([128, NK * BPC], FP32)
        q_t = sb.tile([128, NK * BPC], FP32)
        n32 = sb.tile([128, NK * BPC], FP32)
        d_t = sb.tile([128, NK * BPC], FP32)
        e_t = sb.tile([128, NK * BPC], FP32)
        f_t = sb.tile([128, NK * BPC], FP32)
        g2 = sb.tile([128, NK * BPC], FP32)

        hT_dr = dr.tile([H, TOK], FP16)
        ag_out = dr.tile([NCORES * H, TOK], FP16, addr_space="Shared")
        emb16 = sb.tile([S, BPC, H], FP16)

        # embedding gather (outside the timing loop: indirect DMA does not
        # compile inside For_i in this container)
        for b in range(BPC):
            nc.gpsimd.indirect_dma_start(
                out=emb16[:, b, :],
                out_offset=None,
                in_=emb[:],
                in_offset=bass.IndirectOffsetOnAxis(ap=idx_sb[:, b : b + 1], axis=0),
            )

        def chunk_a():
            # ---- phase 1: transpose + xg GEMM ----
            with tc.tile_pool(name="p1sb", bufs=1) as p1, tc.tile_pool(
                name="p1ps", bufs=2, space="PSUM"
            ) as ps1, tc.tile_pool(name="ptps", bufs=2, space="PSUM") as pst:
                embT = p1.tile([128, NK, TOK], FP16)
                for k in range(NK):
                    for b in range(BPC):
                        p_t = pst.tile([128, 128], FP16)
                        nc.tensor.transpose(
                            p_t[:], emb16[:, b, k * 128 : (k + 1) * 128], ident[:]
                        )
                        nc.vector.tensor_copy(
                            out=embT[:, k, b * S : (b + 1) * S], in_=p_t[:]
                        )
                for m in range(NM):
                    pxg = ps1.tile([128, TOK], FP32)
                    for k in range(NK):
                        nc.tensor.matmul(
                            pxg[:],
                            wih_sb[:, k, m, :],
                            embT[:, k, :],
                            start=(k == 0),
                            stop=(k == NK - 1),
                        )
                    nc.vector.tensor_scalar_add(
                        out=xgT[:, m, :, :], in0=pxg[:], scalar1=bcomb_sb[:, m : m + 1]
                    )

            # ---- phase 2: GRU recurrence ----
            nc.vector.tensor_copy(out=hid32[:, :, :, 0], in_=h0_sb[:])
            nc.vector.tensor_copy(out=hid16[:, :, :, 0], in_=h0_sb[:])
            G = NK * BPC  # 16 gate columns per gate type
            with tc.tile_pool(name="p2ps", bufs=2, space="PSUM") as ps2:
                for s in range(nsteps):
                    # separate psum tiles so the rz gate math can start
                    # while the n-gate matmuls still run on the PE
                    pr_rz = ps2.tile([128, 2 * G], FP32)
                    pr_n = ps2.tile([128, G], FP32)
                    for m in range(8):
                        for k in range(NK):
                            nc.tensor.matmul(
                                pr_rz[:, m * BPC : (m + 1) * BPC],
                                whh_sb[:, k, m, :],
                                hid16[:, k, :, s],
                                start=(k == 0),
                                stop=(k == NK - 1),
                            )
                    for m in range(8, NM):
                        for k in range(NK):
                            nc.tensor.matmul(
                                pr_n[:, (m - 8) * BPC : (m - 7) * BPC],
                                whh_sb[:, k, m, :],
                                hid16[:, k, :, s],
                                start=(k == 0),
                                stop=(k == NK - 1),
                            )
                    # tau = tanh(0.5*(hg_rz + xg_rz))  (sigmoid via tanh)
                    nc.vector.tensor_tensor(
                        out=d1[:], in0=pr_rz[:], in1=xgT[:, 0:8, :, s],
                        op=mybir.AluOpType.add,
                    )
                    nc.scalar.activation(
                        tau[:], d1[:], mybir.ActivationFunctionType.Tanh, scale=0.5
                    )
                    # hnb = hg_n + b_hh_n
                    nc.vector.tensor_tensor(
                        out=hnb[:], in0=pr_n[:], in1=bhn_sb[:],
                        op=mybir.AluOpType.add,
                    )
                    # n = tanh(xn + r*hnb); r = (1+tau_r)/2; xg_n pre-doubled
                    nc.vector.tensor_tensor(
                        out=u_t[:], in0=tau[:, 0:G], in1=hnb[:],
                        op=mybir.AluOpType.mult,
                    )
                    nc.vector.tensor_tensor(
                        out=v_t[:], in0=hnb[:], in1=u_t[:], op=mybir.AluOpType.add
                    )
                    nc.vector.tensor_tensor(
                        out=q_t[:], in0=v_t[:], in1=xgT[:, 8:12, :, s],
                        op=mybir.AluOpType.add,
                    )
                    nc.scalar.activation(
                        n32[:], q_t[:], mybir.ActivationFunctionType.Tanh, scale=0.5
                    )
                    # h' = 0.5*((n+h) + tau_z*(h-n)); z = (1+tau_z)/2
                    nc.vector.tensor_tensor(
                        out=d_t[:], in0=hid32[:, :, :, s], in1=n32[:],
                        op=mybir.AluOpType.subtract,
                    )
                    nc.vector.tensor_tensor(
                        out=e_t[:], in0=tau[:, G : 2 * G], in1=d_t[:],
                        op=mybir.AluOpType.mult,
                    )
                    nc.vector.tensor_tensor(
                        out=f_t[:], in0=n32[:], in1=hid32[:, :, :, s],
                        op=mybir.AluOpType.add,
                    )
                    nc.vector.tensor_tensor(
                        out=g2[:], in0=e_t[:], in1=f_t[:], op=mybir.AluOpType.add
                    )
                    nc.vector.tensor_scalar_mul(
                        out=hid32[:, :, :, s + 1], in0=g2[:], scalar1=0.5
                    )
                    nc.vector.tensor_scalar_mul(
                        out=hid16[:, :, :, s + 1], in0=g2[:], scalar1=0.5
                    )

            # ---- phase 3a: hiddens to DRAM ----
            for k in range(NK):
                nc.sync.dma_start(
                    out=hT_dr[k * 128 : (k + 1) * 128, :],
                    in_=hid16[:, k, :, 1 : S + 1],
                )

        def do_ag():
            # outside the timing loop: collectives do not compile in For_i
            nc.gpsimd.collective_compute(
                "AllGather",
                mybir.AluOpType.bypass,
                ins=[hT_dr[:]],
                outs=[ag_out[:]],
                replica_groups=[list(range(NCORES))],
            )

        def chunk_b():
            # ---- phase 4: output GEMM (vocab shard) ----
            with tc.tile_pool(name="p4sb", bufs=2) as p4, tc.tile_pool(
                name="p4ps", bufs=2, space="PSUM"
            ) as ps4, tc.tile_pool(name="p4out", bufs=2) as p4o:
                for tcn in range(B):
                    src_c, b_loc = tcn // BPC, tcn % BPC
                    lh = p4.tile([128, NK, 128], FP16)
                    for k in range(NK):
                        nc.sync.dma_start(
                            out=lh[:, k, :],
                            in_=ag_out[
                                src_c * H + k * 128 : src_c * H + (k + 1) * 128,
                                b_loc * S : (b_loc + 1) * S,
                            ],
                        )
                    out_sb = p4o.tile([128, VPC], FP32)
                    for half in range(VPC // VHALF):
                        # [128, n, 512] so each matmul region sits in one
                        # 2KB psum bank (regions crossing banks corrupt).
                        po = ps4.tile([128, NVN, 512], FP32)
                        for k in range(NK):
                            for n in range(NVN):
                                nc.tensor.matmul(
                                    po[:, n, 0:500],
                                    lh[:, k, :],
                                    wout_sb[
                                        :, k,
                                        half * VHALF + n * 500 : half * VHALF + (n + 1) * 500,
                                    ],
                                    start=(k == 0),
                                    stop=(k == NK - 1),
                                )
                        nc.vector.tensor_tensor(
                            out=out_sb[:, half * VHALF : (half + 1) * VHALF],
                            in0=po[:, :, 0:500],
                            in1=brep_sb[:, half * VHALF : (half + 1) * VHALF],
                            op=mybir.AluOpType.add,
                        )
                    # timing mode keeps full device DMA volume but rotates
                    # through 2 slots so the host download stays tiny
                    row0 = (tcn % 2) * 128 if timing else tcn * 128
                    nc.sync.dma_start(out=out[row0 : row0 + 128, :], in_=out_sb[:])

        chunk_a()
        do_ag()
        chunk_b()
        if reps > 1:
            # timing loop: repeats everything except gather + AllGather
            # (ag_out is identical every rep, so output stays correct)
            with tc.For_i(0, reps - 1):
                chunk_a()
                chunk_b()

    return nc


def _prep_host(inputs, hidden_init, emb, W_ih, W_hh, b_ih, b_hh, W_out, b_out):
    """Shared + per-core host-side input prep."""
    W_ih_s = W_ih.copy()
    W_ih_s[2 * H :] *= 2.0  # fold the 2x for n = tanh(0.5*(v + 2*xn))
    wih_host = np.ascontiguousarray(
        W_ih_s.reshape(NM, 128, NK, 128).transpose(3, 2, 0, 1).reshape(128, -1)
    ).astype(np.float16)
    whh_host = np.ascontiguousarray(
        W_hh.reshape(NM, 128, NK, 128).transpose(3, 2, 0, 1).reshape(128, -1)
    ).astype(np.float16)
    b_comb = np.concatenate([b_ih[: 2 * H] + b_hh[: 2 * H], 2.0 * b_ih[2 * H :]])
    bcomb_host = np.ascontiguousarray(b_comb.reshape(NM, 128).T)
    bhn_host = np.ascontiguousarray(
        np.repeat(b_hh[2 * H :].reshape(NK, 128).T, BPC, axis=1)
    )
    emb_host = np.ascontiguousarray(emb).astype(np.float16)

    in_maps = []
    for c in range(NCORES):
        bs = slice(BPC * c, BPC * (c + 1))
        vs = slice(VPC * c, VPC * (c + 1))
        idx_t = np.ascontiguousarray(inputs[bs].T).astype(np.int32)
        h0_t = np.ascontiguousarray(
            hidden_init[bs].reshape(BPC, NK, 128).transpose(2, 1, 0).reshape(128, -1)
        )
        wout_host = np.ascontiguousarray(
            W_out[vs].T.reshape(NK, 128, VPC).transpose(1, 0, 2).reshape(128, -1)
        ).astype(np.float16)
        brep_host = np.ascontiguousarray(
            np.broadcast_to(b_out[vs][None, :], (128, VPC))
        )
        in_maps.append(
            {
                "idx_t": idx_t,
                "emb": emb_host,
                "h0_t": h0_t,
                "w_ih": wih_host,
                "w_hh": whh_host,
                "b_comb": bcomb_host,
                "b_hn": bhn_host,
                "w_out": wout_host,
                "b_rep": brep_host,
            }
        )
    return in_maps


def run_on_cores(nc, in_maps):
    _install_syncfix()
    from concourse.bass_utils import run_bass_kernel_spmd

    res = run_bass_kernel_spmd(nc, in_maps, core_ids=list(range(NCORES)))
    return res.results


_nc_cache = {}


def kernel(**inputs) -> np.ndarray:
    in_maps = _prep_host(
        inputs["inputs"].astype(np.int32),
        np.asarray(inputs["hidden_init"], np.float32),
        np.asarray(inputs["emb"], np.float32),
        np.asarray(inputs["W_ih"], np.float32),
        np.asarray(inputs["W_hh"], np.float32),
        np.asarray(inputs["b_ih"], np.float32),
        np.asarray(inputs["b_hh"], np.float32),
        np.asarray(inputs["W_out"], np.float32),
        np.asarray(inputs["b_out"], np.float32),
    )
    if "nc" not in _nc_cache:
        _nc_cache["nc"] = build_nc()
    results = run_on_cores(_nc_cache["nc"], in_maps)
    full = np.empty((B, S, V), np.float32)
    for c in range(NCORES):
        full[:, :, VPC * c : VPC * (c + 1)] = results[c]["out"].reshape(B, S, VPC)
    return full


# revision 14
# speedup vs baseline: 1.2585x; 1.2585x over previous
"""Trainium2 Bass kernel for GRU decoder (nn_RNNDecoder).

B=32, S=128, H=512, V=32000. Sharding: data-parallel recurrence (4
batches/core) + vocab-parallel output GEMM (4000 rows/core) with an
AllGather of the hidden states in between. All GEMMs run in fp16 with
fp32 psum accumulation and an fp32 master copy of h.
"""

import sys

sys.path.insert(0, "/opt/trn_rl_repo")

import json as _json
from contextlib import ExitStack
from itertools import count as _count

import numpy as np

import concourse.bass as bass
import concourse.tile as tile
from concourse import mybir
from concourse.masks import make_identity

FP32 = mybir.dt.float32
FP16 = mybir.dt.float16
I32 = mybir.dt.int32

B, S, H, V = 32, 128, 512, 32000
NCORES = 8
BPC = B // NCORES  # batches per core
VPC = V // NCORES  # vocab rows per core
TOK = BPC * S  # tokens per core
NK = H // 128  # 4 hidden chunks
NM = 3 * H // 128  # 12 gate chunks
VHALF = 2000  # vocab per psum group
NVN = 4  # psum tiles per group ([128,500] each)

# ---------------------------------------------------------------------------
# Workaround for this container's walrus codegen: instructions whose ISA
# struct has fewer sync-wait slots than the tile framework emits fail with
# "Too many sync wait commands". Split excess waits onto NoOp carriers on
# the same engine (in-order execution preserves semantics).
_uid = _count()


def _fix_bir_sync(bir_json, limit=1):
    m = _json.loads(bir_json)
    for fn in m["functions"]:
        for blk in fn["blocks"]:
            out = []
            for inst in blk["instructions"]:
                si = inst.get("sync_info") or {}
                waits = si.get("on_wait") or []
                if len(waits) > limit:
                    keep = waits[len(waits) - limit :]
                    excess = waits[: len(waits) - limit]
                    for w in excess:
                        out.append(
                            {
                                "engine": inst["engine"],
                                "ins": [],
                                "outs": [],
                                "name": f"syncfix-{next(_uid)}",
                                "opcode": "NoOp",
                                "sync_info": {"on_update": [], "on_wait": [w]},
                            }
                        )
                    si = dict(si)
                    si["on_wait"] = keep
                    inst["sync_info"] = si
                out.append(inst)
            blk["instructions"] = out
    return _json.dumps(m).encode()


_installed = False


def _install_syncfix():
    global _installed
    if _installed:
        return
    from concourse import bass_utils, bass2jax

    orig = bass_utils.compile_bir_kernel

    def patched(bir_json, tmpdir, neff_name="file.neff"):
        return orig(_fix_bir_sync(bir_json), tmpdir, neff_name)

    bass_utils.compile_bir_kernel = patched
    bass2jax.compile_bir_kernel = patched
    _installed = True


# ---------------------------------------------------------------------------


def build_nc(nsteps=S, reps=1, timing=False, loop_phase="ab"):
    # timing=True shrinks the host-transfer surface (tiny emb upload, tiny
    # output download) without changing device-side work in the timed loop.
    nc = bass.Bass()
    idx_t = nc.declare_dram_parameter("idx_t", [S, BPC], I32, isOutput=False)
    emb = nc.declare_dram_parameter("emb", [128 if timing else V, H], FP16, isOutput=False)
    h0_t = nc.declare_dram_parameter("h0_t", [128, NK * BPC], FP32, isOutput=False)
    w_ih = nc.declare_dram_parameter("w_ih", [128, NK * NM * 128], FP16, isOutput=False)
    w_hh = nc.declare_dram_parameter("w_hh", [128, NK * NM * 128], FP16, isOutput=False)
    b_comb = nc.declare_dram_parameter("b_comb", [128, NM], FP32, isOutput=False)
    b_hn = nc.declare_dram_parameter("b_hn", [128, NK * BPC], FP32, isOutput=False)
    w_out = nc.declare_dram_parameter("w_out", [128, NK * VPC], FP16, isOutput=False)
    b_rep = nc.declare_dram_parameter("b_rep", [128, VPC], FP32, isOutput=False)
    out = nc.declare_dram_parameter(
        "out", [256 if timing else B * S, VPC], FP32, isOutput=True
    )

    with tile.TileContext(nc) as tc, ExitStack() as ctx:
        sb = ctx.enter_context(tc.tile_pool(name="sb", bufs=1))
        dr = ctx.enter_context(tc.tile_pool(name="dr", bufs=1, space="DRAM"))

        # persistent SBUF state
        idx_sb = sb.tile([S, BPC], I32)
        h0_sb = sb.tile([128, NK * BPC], FP32)
        wih_sb = sb.tile([128, NK, NM, 128], FP16)
        whh_sb = sb.tile([128, NK, NM, 128], FP16)
        bcomb_sb = sb.tile([128, NM], FP32)
        bhn_sb = sb.tile([128, NK * BPC], FP32)
        wout_sb = sb.tile([128, NK, VPC], FP16)
        brep_sb = sb.tile([128, VPC], FP32)
        nc.sync.dma_start(out=idx_sb[:], in_=idx_t[:])
        nc.sync.dma_start(out=h0_sb[:], in_=h0_t[:])
        nc.sync.dma_start(out=wih_sb[:], in_=w_ih[:])
        nc.sync.dma_start(out=whh_sb[:], in_=w_hh[:])
        nc.sync.dma_start(out=bcomb_sb[:], in_=b_comb[:])
        nc.sync.dma_start(out=bhn_sb[:], in_=b_hn[:])
        nc.sync.dma_start(out=wout_sb[:], in_=w_out[:])
        nc.sync.dma_start(out=brep_sb[:], in_=b_rep[:])

        ident = sb.tile([128, 128], FP16)
        make_identity(nc, ident[:])

        xgT = sb.tile([128, NM, BPC, S], FP32)
        hid32 = sb.tile([128, NK, BPC, S + 1], FP32)
        hid16 = sb.tile([128, NK, BPC, S + 1], FP16)

        # gate-math temporaries (reused every step)
        d1 = sb.tile([128, 2 * NK * BPC], FP32)
        tau = sb.tile([128, 2 * NK * BPC], FP32)
        hnb = sb.tile([128, NK * BPC], FP32)
        u_t = sb.tile([128, NK * BPC], FP32)
        v_t = sb.tile([128, NK * BPC], FP32)
        q_t = sb.tile([128, NK * BPC], FP32)
        n32 = sb.tile([128, NK * BPC], FP32)
        d_t = sb.tile([128, NK * BPC], FP32)
        e_t = sb.tile([128, NK * BPC], FP32)
        f_t = sb.tile([128, NK * BPC], FP32)
        g2 = sb.tile([128, NK * BPC], FP32)

        hT_dr = dr.tile([H, TOK], FP16)
        ag_out = dr.tile([NCORES * H, TOK], FP16, addr_space="Shared")
        emb16 = sb.tile([S, BPC, H], FP16)

        # embedding gather (outside the timing loop: indirect DMA does not
        # compile inside For_i in this container)
        for b in range(BPC):
            nc.gpsimd.indirect_dma_start(
                out=emb16[:, b, :],
                out_offset=None,
                in_=emb[:],
                in_offset=bass.IndirectOffsetOnAxis(ap=idx_sb[:, b : b + 1], axis=0),
            )

        def chunk_a():
            # ---- phase 1: transpose + xg GEMM ----
            with tc.tile_pool(name="p1sb", bufs=1) as p1, tc.tile_pool(
                name="p1ps", bufs=2, space="PSUM"
            ) as ps1, tc.tile_pool(name="ptps", bufs=2, space="PSUM") as pst:
                embT = p1.tile([128, NK, TOK], FP16)
                for k in range(NK):
                    for b in range(BPC):
                        p_t = pst.tile([128, 128], FP16)
                        nc.tensor.transpose(
                            p_t[:], emb16[:, b, k * 128 : (k + 1) * 128], ident[:]
                        )
                        nc.vector.tensor_copy(
                            out=embT[:, k, b * S : (b + 1) * S], in_=p_t[:]
                        )
                for m in range(NM):
                    pxg = ps1.tile([128, TOK], FP32)
                    for k in range(NK):
                        nc.tensor.matmul(
                            pxg[:],
                            wih_sb[:, k, m, :],
                            embT[:, k, :],
                            start=(k == 0),
                            stop=(k == NK - 1),
                        )
                    nc.vector.tensor_scalar_add(
                        out=xgT[:, m, :, :], in0=pxg[:], scalar1=bcomb_sb[:, m : m + 1]
                    )

            # ---- phase 2: GRU recurrence ----
            nc.vector.tensor_copy(out=hid32[:, :, :, 0], in_=h0_sb[:])
            nc.vector.tensor_copy(out=hid16[:, :, :, 0], in_=h0_sb[:])
            G = NK * BPC  # 16 gate columns per gate type
            with tc.tile_pool(name="p2ps", bufs=2, space="PSUM") as ps2:
                for s in range(nsteps):
                    pr = ps2.tile([128, 3 * G], FP32)
                    for m in range(NM):
                        for k in range(NK):
                            nc.tensor.matmul(
                                pr[:, m * BPC : (m + 1) * BPC],
                                whh_sb[:, k, m, :],
                                hid16[:, k, :, s],
                                start=(k == 0),
                                stop=(k == NK - 1),
                            )
                    # tau = tanh(0.5*(hg_rz + xg_rz))  (sigmoid via tanh)
                    nc.vector.tensor_tensor(
                        out=d1[:], in0=pr[:, 0 : 2 * G], in1=xgT[:, 0:8, :, s],
                        op=mybir.AluOpType.add,
                    )
                    nc.scalar.activation(
                        tau[:], d1[:], mybir.ActivationFunctionType.Tanh, scale=0.5
                    )
                    # hnb = hg_n + b_hh_n
                    nc.vector.tensor_tensor(
                        out=hnb[:], in0=pr[:, 2 * G : 3 * G], in1=bhn_sb[:],
                        op=mybir.AluOpType.add,
                    )
                    # n = tanh(xn + r*hnb); r = (1+tau_r)/2; xg_n pre-doubled
                    nc.vector.tensor_tensor(
                        out=u_t[:], in0=tau[:, 0:G], in1=hnb[:],
                        op=mybir.AluOpType.mult,
                    )
                    nc.vector.tensor_tensor(
                        out=v_t[:], in0=hnb[:], in1=u_t[:], op=mybir.AluOpType.add
                    )
                    nc.vector.tensor_tensor(
                        out=q_t[:], in0=v_t[:], in1=xgT[:, 8:12, :, s],
                        op=mybir.AluOpType.add,
                    )
                    nc.scalar.activation(
                        n32[:], q_t[:], mybir.ActivationFunctionType.Tanh, scale=0.5
                    )
                    # h' = 0.5*((n+h) + tau_z*(h-n)); z = (1+tau_z)/2
                    nc.vector.tensor_tensor(
                        out=d_t[:], in0=hid32[:, :, :, s], in1=n32[:],
                        op=mybir.AluOpType.subtract,
                    )
                    nc.vector.tensor_tensor(
                        out=e_t[:], in0=tau[:, G : 2 * G], in1=d_t[:],
                        op=mybir.AluOpType.mult,
                    )
                    nc.vector.tensor_tensor(
                        out=f_t[:], in0=n32[:], in1=hid32[:, :, :, s],
                        op=mybir.AluOpType.add,
                    )
                    nc.vector.tensor_tensor(
                        out=g2[:], in0=e_t[:], in1=f_t[:], op=mybir.AluOpType.add
                    )
                    nc.vector.tensor_scalar_mul(
                        out=hid32[:, :, :, s + 1], in0=g2[:], scalar1=0.5
                    )
                    nc.vector.tensor_scalar_mul(
                        out=hid16[:, :, :, s + 1], in0=g2[:], scalar1=0.5
                    )

            # ---- phase 3a: hiddens to DRAM ----
            for k in range(NK):
                nc.sync.dma_start(
                    out=hT_dr[k * 128 : (k + 1) * 128, :],
                    in_=hid16[:, k, :, 1 : S + 1],
                )

        def do_ag():
            # outside the timing loop: collectives do not compile in For_i
            nc.gpsimd.collective_compute(
                "AllGather",
                mybir.AluOpType.bypass,
                ins=[hT_dr[:]],
                outs=[ag_out[:]],
                replica_groups=[list(range(NCORES))],
            )

        def chunk_b():
            # ---- phase 4: output GEMM (vocab shard) ----
            with tc.tile_pool(name="p4sb", bufs=2) as p4, tc.tile_pool(
                name="p4ps", bufs=2, space="PSUM"
            ) as ps4, tc.tile_pool(name="p4out", bufs=2) as p4o:
                for tcn in range(B):
                    src_c, b_loc = tcn // BPC, tcn % BPC
                    lh = p4.tile([128, NK, 128], FP16)
                    for k in range(NK):
                        nc.sync.dma_start(
                            out=lh[:, k, :],
                            in_=ag_out[
                                src_c * H + k * 128 : src_c * H + (k + 1) * 128,
                                b_loc * S : (b_loc + 1) * S,
                            ],
                        )
                    out_sb = p4o.tile([128, VPC], FP32)
                    for half in range(VPC // VHALF):
                        # [128, n, 512] so each matmul region sits in one
                        # 2KB psum bank (regions crossing banks corrupt).
                        po = ps4.tile([128, NVN, 512], FP32)
                        for k in range(NK):
                            for n in range(NVN):
                                nc.tensor.matmul(
                                    po[:, n, 0:500],
                                    lh[:, k, :],
                                    wout_sb[
                                        :, k,
                                        half * VHALF + n * 500 : half * VHALF + (n + 1) * 500,
                                    ],
                                    start=(k == 0),
                                    stop=(k == NK - 1),
                                )
                        nc.vector.tensor_tensor(
                            out=out_sb[:, half * VHALF : (half + 1) * VHALF],
                            in0=po[:, :, 0:500],
                            in1=brep_sb[:, half * VHALF : (half + 1) * VHALF],
                            op=mybir.AluOpType.add,
                        )
                    # timing mode keeps full device DMA volume but rotates
                    # through 2 slots so the host download stays tiny
                    row0 = (tcn % 2) * 128 if timing else tcn * 128
                    nc.sync.dma_start(out=out[row0 : row0 + 128, :], in_=out_sb[:])

        chunk_a()
        do_ag()
        chunk_b()
        if reps > 1:
            # timing loop: repeats everything except gather + AllGather
            # (ag_out is identical every rep, so output stays correct)
            with tc.For_i(0, reps - 1):
                if "a" in loop_phase:
                    chunk_a()
                if "b" in loop_phase:
                    chunk_b()

    return nc


def _prep_host(inputs, hidden_init, emb, W_ih, W_hh, b_ih, b_hh, W_out, b_out):
    """Shared + per-core host-side input prep."""
    W_ih_s = W_ih.copy()
    W_ih_s[2 * H :] *= 2.0  # fold the 2x for n = tanh(0.5*(v + 2*xn))
    wih_host = np.ascontiguousarray(
        W_ih_s.reshape(NM, 128, NK, 128).transpose(3, 2, 0, 1).reshape(128, -1)
    ).astype(np.float16)
    whh_host = np.ascontiguousarray(
        W_hh.reshape(NM, 128, NK, 128).transpose(3, 2, 0, 1).reshape(128, -1)
    ).astype(np.float16)
    b_comb = np.concatenate([b_ih[: 2 * H] + b_hh[: 2 * H], 2.0 * b_ih[2 * H :]])
    bcomb_host = np.ascontiguousarray(b_comb.reshape(NM, 128).T)
    bhn_host = np.ascontiguousarray(
        np.repeat(b_hh[2 * H :].reshape(NK, 128).T, BPC, axis=1)
    )
    emb_host = np.ascontiguousarray(emb).astype(np.float16)

    in_maps = []
    for c in range(NCORES):
        bs = slice(BPC * c, BPC * (c + 1))
        vs = slice(VPC * c, VPC * (c + 1))
        idx_t = np.ascontiguousarray(inputs[bs].T).astype(np.int32)
        h0_t = np.ascontiguousarray(
            hidden_init[bs].reshape(BPC, NK, 128).transpose(2, 1, 0).reshape(128, -1)
        )
        wout_host = np.ascontiguousarray(
            W_out[vs].T.reshape(NK, 128, VPC).transpose(1, 0, 2).reshape(128, -1)
        ).astype(np.float16)
        brep_host = np.ascontiguousarray(
            np.broadcast_to(b_out[vs][None, :], (128, VPC))
        )
        in_maps.append(
            {
                "idx_t": idx_t,
                "emb": emb_host,
                "h0_t": h0_t,
                "w_ih": wih_host,
                "w_hh": whh_host,
                "b_comb": bcomb_host,
                "b_hn": bhn_host,
                "w_out": wout_host,
                "b_rep": brep_host,
            }
        )
    return in_maps


def run_on_cores(nc, in_maps):
    _install_syncfix()
    from concourse.bass_utils import run_bass_kernel_spmd

    res = run_bass_kernel_spmd(nc, in_maps, core_ids=list(range(NCORES)))
    return res.results


_nc_cache = {}


def kernel(**inputs) -> np.ndarray:
    in_maps = _prep_host(
        inputs["inputs"].astype(np.int32),
        np.asarray(inputs["hidden_init"], np.float32),
        np.asarray(inputs["emb"], np.float32),
        np.asarray(inputs["W_ih"], np.float32),
        np.asarray(inputs["W_hh"], np.float32),
        np.asarray(inputs["b_ih"], np.float32),
        np.asarray(inputs["b_hh"], np.float32),
        np.asarray(inputs["W_out"], np.float32),
        np.asarray(inputs["b_out"], np.float32),
    )
    if "nc" not in _nc_cache:
        _nc_cache["nc"] = build_nc()
    results = run_on_cores(_nc_cache["nc"], in_maps)
    full = np.empty((B, S, V), np.float32)
    for c in range(NCORES):
        full[:, :, VPC * c : VPC * (c + 1)] = results[c]["out"].reshape(B, S, VPC)
    return full
